# revision 2
# baseline (speedup 1.0000x reference)
"""CrossModalAttention Trainium2 kernel (v2).

Full inputs in, full outputs out; internally sharded data-parallel over the
batch dim across 8 NeuronCores (4 batch items per core).

Per batch item (C=256, H=W=64, AS=8, T=64):
  - Host pre-casts F_d -> fp8e4m3 (quarter DMA read bytes) and
    F_rgb -> (1-alpha)*F_rgb in fp16; both pre-transposed to [128, ci*HW]
    layout so device DMAs are fully contiguous.
  - D h-pool: 4 fp8 DoubleRow identity-matmuls per channel chunk on PE
    (pairs of h-rows per pass, accumulating in PSUM), ACT copy -> fp16,
    then a 3-level w-direction pairwise-add tree on DVE (both chunks fused
    per op for fewer instruction overheads).
  - R pool: 6-level pairwise-add tree on DVE (h-direction first so every
    level is unit-stride / 2x-mode eligible; both chunks fused per op).
  - Q = Wq@R+bq, K = Wk@D+bk as [o, s] (PE fp16 + ACT bias, fp16 out);
    the 1/64 pool mean and the 1/(1-alpha) unfold ride the host-folded
    weights.
  - VfT = D^T @ Wv^T + ones^T@bv as [s, o] (PE fp16)
  - A = Qf^T Kf [t, s] fp32 PSUM; exp without max-subtraction (logits are
    O(1)); row-sum + reciprocal on DVE; normalization is folded into the
    PSUM->SBUF copy of Fatt^T via ACT per-partition scale.
  - E^T via PE transpose; FattT = E^T @ VfT [t, c] (PE fp16)
  - upsample: psum = FattT_chunk^T @ (alpha*kron(U^T,U^T))_block per
    512-wide block (PE fp16); ACT copies psum -> fp16 out tile; DVE adds
    the pre-scaled F_rgb chunk in one 4096-wide fp16 2x-mode add (the
    blend); one DMA store per (batch, chunk); host upcasts to fp32.
"""

import numpy as np
import ml_dtypes
from contextlib import ExitStack

import concourse.bacc as bacc
import concourse.mybir as mybir
import concourse.tile as tile
from concourse.bass_utils import run_bass_kernel_spmd

B, C, H, W = 32, 256, 64, 64
AS = 8
T = AS * AS          # 64 pooled pixels
HW = H * W           # 4096
NCORES = 8
BPC = B // NCORES    # 4 batch items per core
NCHUNK = C // 128    # 2 channel chunks

F32 = mybir.dt.float32
F16 = mybir.dt.float16
F8 = mybir.dt.float8e4
NPF16 = np.float16
NPF8 = ml_dtypes.float8_e4m3
DR = mybir.MatmulPerfMode.DoubleRow


def _bilinear_up_matrix(n_out: int, n_in: int) -> np.ndarray:
    """U[i, p]: weight of coarse pixel p for fine pixel i; half-pixel centers
    with edge clamping (identical to jax.image.resize bilinear upsample)."""
    U = np.zeros((n_out, n_in), np.float64)
    scale = n_in / n_out
    for i in range(n_out):
        src = (i + 0.5) * scale - 0.5
        p0 = int(np.floor(src))
        f = src - p0
        for p, wgt in ((p0, 1.0 - f), (p0 + 1, f)):
            pc = min(max(p, 0), n_in - 1)
            U[i, pc] += wgt
    return U


_CACHE = {}


def _build_program(blend: bool):
    nc = bacc.Bacc("TRN2", target_bir_lowering=False, debug=False,
                   num_devices=NCORES)

    frgb = nc.dram_tensor("frgb", [BPC, 128, NCHUNK * HW], F16,
                          kind="ExternalInput").ap()
    fd = nc.dram_tensor("fd", [BPC, 128, NCHUNK * HW], F8,
                        kind="ExternalInput").ap()
    wqt = nc.dram_tensor("wqt", [128, NCHUNK * C], F16, kind="ExternalInput").ap()
    wkt = nc.dram_tensor("wkt", [128, NCHUNK * C], F16, kind="ExternalInput").ap()
    wvt = nc.dram_tensor("wvt", [128, NCHUNK * C], F16, kind="ExternalInput").ap()
    bq2 = nc.dram_tensor("bq2", [128, NCHUNK], F32, kind="ExternalInput").ap()
    bk2 = nc.dram_tensor("bk2", [128, NCHUNK], F32, kind="ExternalInput").ap()
    bvr = nc.dram_tensor("bvr", [1, C], F16, kind="ExternalInput").ap()
    u2a = nc.dram_tensor("u2a", [T, HW], F16, kind="ExternalInput").ap()
    id64 = nc.dram_tensor("id64", [T, T], F16, kind="ExternalInput").ap()
    ones64 = nc.dram_tensor("ones64", [1, T], F16, kind="ExternalInput").ap()
    id2 = nc.dram_tensor("id2", [128, 2 * 128], F8, kind="ExternalInput").ap()
    out = nc.dram_tensor("out", [BPC, NCHUNK, 128, HW], F16,
                         kind="ExternalOutput").ap()

    with tile.TileContext(nc) as tc, ExitStack() as ctx:
        consts = ctx.enter_context(tc.tile_pool(name="consts", bufs=1))
        fr_pool = ctx.enter_context(tc.tile_pool(name="fr", bufs=3))
        fd_pool = ctx.enter_context(tc.tile_pool(name="fdp", bufs=3))
        out_pool = ctx.enter_context(tc.tile_pool(name="outp", bufs=4))
        scr_pool = ctx.enter_context(tc.tile_pool(name="scr", bufs=2))
        small = ctx.enter_context(tc.tile_pool(name="small", bufs=2))
        ps_d = ctx.enter_context(
            tc.tile_pool(name="psd", bufs=2, space="PSUM"))
        ps_small = ctx.enter_context(
            tc.tile_pool(name="pss", bufs=2, space="PSUM"))
        ps_out = ctx.enter_context(
            tc.tile_pool(name="pso", bufs=2, space="PSUM"))

        # ---- constants into SBUF ----
        wqt_s = consts.tile([128, NCHUNK * C], F16)   # [c, (ci, o)]
        nc.sync.dma_start(wqt_s[:], wqt[:])
        wkt_s = consts.tile([128, NCHUNK * C], F16)
        nc.sync.dma_start(wkt_s[:], wkt[:])
        wvt_s = consts.tile([128, NCHUNK * C], F16)
        nc.sync.dma_start(wvt_s[:], wvt[:])
        bq_s = consts.tile([128, NCHUNK], F32)
        nc.sync.dma_start(bq_s[:], bq2[:])
        bk_s = consts.tile([128, NCHUNK], F32)
        nc.sync.dma_start(bk_s[:], bk2[:])
        bvr_s = consts.tile([1, C], F16)
        nc.sync.dma_start(bvr_s[:], bvr[:])
        u2a_s = consts.tile([T, HW], F16)
        nc.sync.dma_start(u2a_s[:], u2a[:])
        id64_s = consts.tile([T, T], F16)
        nc.sync.dma_start(id64_s[:], id64[:])
        ones_s = consts.tile([1, T], F16)
        nc.sync.dma_start(ones_s[:], ones64[:])
        id2_s = consts.tile([128, 2 * 128], F8)
        nc.sync.dma_start(id2_s[:], id2[:])
        id2_v = id2_s.rearrange("p (i m) -> p i m", i=2)

        for b in range(BPC):
            # ---- load (1-a)*F_rgb (fp16) and F_d (fp8), host pre-cast ----
            fr_t = fr_pool.tile([128, NCHUNK * HW], F16, tag="fr")
            nc.sync.dma_start(fr_t[:], frgb[b])
            fd_t = fd_pool.tile([128, NCHUNK * HW], F8, tag="fd")
            nc.sync.dma_start(fd_t[:], fd[b])

            # ---- D h-pool: 4 fp8 DoubleRow id-matmuls per chunk on PE ----
            # free index = ci*4096 + a*512 + k*128 + v*64 + w  (h = a*8+k*2+v)
            fdv = fd_t.rearrange("p (ci a k v w) -> p ci k v a w",
                                 ci=NCHUNK, a=AS, k=4, v=2)
            ds1 = small.tile([128, NCHUNK * 512], F16, tag="ds1")
            for ci in range(NCHUNK):
                pd = ps_d.tile([128, 512], F32, tag="pd")
                for k in range(4):
                    nc.tensor.matmul(pd[:], id2_v, fdv[:, ci, k],
                                     start=(k == 0), stop=(k == 3),
                                     perf_mode=DR)
                nc.scalar.copy(ds1[:, ci * 512:(ci + 1) * 512], pd[:])

            # ---- R pool: 6-level DVE tree, both chunks fused per op ----
            # free index = g*512 + v*64 + w with g = ci*8 + a
            xv = fr_t.rearrange("p (g v w) -> p g v w", g=2 * AS, w=W)
            h1 = scr_pool.tile([128, 4096], F16, tag="h1")
            h1v = h1.rearrange("p (g v w) -> p g v w", g=2 * AS, w=W)
            nc.vector.tensor_add(h1v, xv[:, :, 0:4, :], xv[:, :, 4:8, :])
            h2 = scr_pool.tile([128, 2048], F16, tag="h2")
            h2v = h2.rearrange("p (g v w) -> p g v w", g=2 * AS, w=W)
            nc.vector.tensor_add(h2v, h1v[:, :, 0:2, :], h1v[:, :, 2:4, :])
            h3 = scr_pool.tile([128, 1024], F16, tag="h3")
            h3v = h3.rearrange("p (g v w) -> p g v w", g=2 * AS, w=W)
            nc.vector.tensor_add(h3v, h2v[:, :, 0:1, :], h2v[:, :, 1:2, :])
            # w-tree: [p, (g16, s8, u8)]
            h3w = h3.rearrange("p (g s u) -> p g s u", g=2 * AS, u=AS)
            r1t = scr_pool.tile([128, 512], F16, tag="r1t")
            r1v = r1t.rearrange("p (g s u) -> p g s u", g=2 * AS, u=4)
            nc.vector.tensor_add(r1v, h3w[:, :, :, 0:4], h3w[:, :, :, 4:8])
            r2t = scr_pool.tile([128, 256], F16, tag="r2t")
            r2v = r2t.rearrange("p (g s u) -> p g s u", g=2 * AS, u=2)
            nc.vector.tensor_add(r2v, r1v[:, :, :, 0:2], r1v[:, :, :, 2:4])
            rs_t = small.tile([128, NCHUNK * T], F16, tag="rs")
            rsv = rs_t.rearrange("p (g s u) -> p g s u", g=2 * AS, u=1)
            nc.vector.tensor_add(rsv, r2v[:, :, :, 0:1], r2v[:, :, :, 1:2])

            # ---- D w-tree (3 DVE ops, both chunks fused) ----
            dsw = ds1.rearrange("p (g s u) -> p g s u", g=2 * AS, u=AS)
            d1 = scr_pool.tile([128, 512], F16, tag="d1")
            d1v = d1.rearrange("p (g s u) -> p g s u", g=2 * AS, u=4)
            nc.vector.tensor_add(d1v, dsw[:, :, :, 0:4], dsw[:, :, :, 4:8])
            d2 = scr_pool.tile([128, 256], F16, tag="d2")
            d2v = d2.rearrange("p (g s u) -> p g s u", g=2 * AS, u=2)
            nc.vector.tensor_add(d2v, d1v[:, :, :, 0:2], d1v[:, :, :, 2:4])
            ds_t = small.tile([128, NCHUNK * T], F16, tag="ds")
            dsv = ds_t.rearrange("p (g s u) -> p g s u", g=2 * AS, u=1)
            nc.vector.tensor_add(dsv, d2v[:, :, :, 0:1], d2v[:, :, :, 1:2])

            # ---- Q, K: [o, s] with per-partition bias (fp16 out) ----
            qf_t = small.tile([128, NCHUNK * T], F16, tag="qf")
            kf_t = small.tile([128, NCHUNK * T], F16, tag="kf")
            for w_s, b_s, sums, dst in ((wqt_s, bq_s, rs_t, qf_t),
                                        (wkt_s, bk_s, ds_t, kf_t)):
                for oj in range(NCHUNK):
                    psq = ps_small.tile([128, T], F32, tag="pss")
                    for ci in range(NCHUNK):
                        nc.tensor.matmul(
                            psq[:],
                            w_s[:, ci * C + oj * 128: ci * C + (oj + 1) * 128],
                            sums[:, ci * T:(ci + 1) * T],
                            start=(ci == 0), stop=(ci == NCHUNK - 1))
                    nc.scalar.activation(
                        dst[:, oj * T:(oj + 1) * T], psq[:],
                        mybir.ActivationFunctionType.Identity,
                        bias=b_s[:, oj:oj + 1], scale=1.0)

            # ---- VfT = D^T Wv^T + ones^T bv : [s, o] ----
            psv = ps_small.tile([T, C], F32, tag="pss")
            for ci in range(NCHUNK):
                nc.tensor.matmul(psv[:],
                                 ds_t[:, ci * T:(ci + 1) * T],
                                 wvt_s[:, ci * C:(ci + 1) * C],
                                 start=(ci == 0), stop=False)
            nc.tensor.matmul(psv[:], ones_s[:], bvr_s[:], start=False, stop=True)
            vft = small.tile([T, C], F16, tag="vft")
            nc.scalar.copy(vft[:], psv[:])

            # ---- A = Qf^T Kf : [t, s] ----
            psa = ps_small.tile([T, T], F32, tag="pss")
            for oj in range(NCHUNK):
                nc.tensor.matmul(psa[:],
                                 qf_t[:, oj * T:(oj + 1) * T],
                                 kf_t[:, oj * T:(oj + 1) * T],
                                 start=(oj == 0), stop=(oj == NCHUNK - 1))

            # ---- softmax: exp (no max-sub; logits O(1)), sum, recip ----
            e16 = small.tile([T, T], F16, tag="e16")
            nc.scalar.activation(e16[:], psa[:],
                                 mybir.ActivationFunctionType.Exp)
            s1 = small.tile([T, 1], F32, tag="s1")
            nc.vector.reduce_sum(s1[:], e16[:], axis=mybir.AxisListType.X)
            rr = small.tile([T, 1], F32, tag="rr")
            nc.vector.reciprocal(rr[:], s1[:])

            # ---- E^T via PE transpose ----
            psat = ps_small.tile([T, T], F16, tag="pss")
            nc.tensor.transpose(psat[:], e16[:], id64_s[:])
            et = small.tile([T, T], F16, tag="et")
            nc.scalar.copy(et[:], psat[:])

            # ---- FattT = E^T @ VfT : [t, c]; row-normalize on the copy ----
            psf = ps_small.tile([T, C], F32, tag="pss")
            nc.tensor.matmul(psf[:], et[:], vft[:], start=True, stop=True)
            ft = small.tile([T, C], F16, tag="ft")
            nc.scalar.activation(ft[:], psf[:],
                                 mybir.ActivationFunctionType.Identity,
                                 scale=rr[:, 0:1])

            # ---- upsample (PE) -> ACT copy -> one fused blend add -> store
            for ci in range(NCHUNK):
                out_ct = out_pool.tile([128, HW], F16, tag="oc")
                for nb in range(HW // 1024):
                    pso = ps_out.tile([128, 1024], F32, tag="pso")
                    for hb in range(2):
                        nc.tensor.matmul(
                            pso[:, hb * 512:(hb + 1) * 512],
                            ft[:, ci * 128:(ci + 1) * 128],
                            u2a_s[:, nb * 1024 + hb * 512:
                                  nb * 1024 + (hb + 1) * 512],
                            start=True, stop=True)
                    nc.scalar.copy(out_ct[:, nb * 1024:(nb + 1) * 1024],
                                   pso[:])
                if blend:
                    nc.vector.tensor_add(
                        out_ct[:], out_ct[:],
                        fr_t[:, ci * HW:(ci + 1) * HW])
                nc.sync.dma_start(out[b, ci], out_ct[:])

    nc.compile()
    return nc


def _prepare_in_maps(F_rgb, F_d, Wq, bq, Wk, bk, Wv, bv, alpha):
    if "U" not in _CACHE:
        _CACHE["U"] = _bilinear_up_matrix(H, AS)
    U = _CACHE["U"]

    a = float(np.asarray(alpha))
    blend = abs(1.0 - a) > 1e-7
    rscale = (1.0 - a) if blend else 1.0

    F_rgb = (np.asarray(F_rgb, np.float32) * np.float32(rscale)).astype(NPF16)
    F_d = np.asarray(F_d, np.float32).astype(NPF8)

    # [core, b, c_chunk, 128, hw] -> [core, b, 128, (c_chunk, hw)] contiguous
    frgb_sh = np.ascontiguousarray(
        F_rgb.reshape(NCORES, BPC, NCHUNK, 128, HW).transpose(0, 1, 3, 2, 4)
    ).reshape(NCORES, BPC, 128, NCHUNK * HW)
    fd_sh = np.ascontiguousarray(
        F_d.reshape(NCORES, BPC, NCHUNK, 128, HW).transpose(0, 1, 3, 2, 4)
    ).reshape(NCORES, BPC, 128, NCHUNK * HW)

    def wfold(Wx, extra=1.0):
        # [c, (ci, o)] layout of (Wx / 64 / extra)^T
        return np.ascontiguousarray(
            (np.asarray(Wx, np.float64).T / (T * extra))
            .reshape(NCHUNK, 128, C).transpose(1, 0, 2)
        ).reshape(128, NCHUNK * C).astype(NPF16)

    wqt = wfold(Wq, extra=rscale)   # R sums are pre-scaled by rscale
    wkt = wfold(Wk)
    wvt = wfold(Wv)
    bq2 = np.ascontiguousarray(np.asarray(bq, np.float32).reshape(NCHUNK, 128).T)
    bk2 = np.ascontiguousarray(np.asarray(bk, np.float32).reshape(NCHUNK, 128).T)
    bvr = np.asarray(bv, np.float32).reshape(1, C).astype(NPF16)
    u2a = (a * np.kron(U.T, U.T)).astype(NPF16)
    id64 = np.eye(T, dtype=np.float32).astype(NPF16)
    ones64 = np.ones((1, T), NPF16)
    id2 = np.zeros((128, 2, 128), np.float32)
    for i in range(2):
        id2[np.arange(128), i, np.arange(128)] = 1.0
    id2 = id2.reshape(128, 256).astype(NPF8)

    in_maps = []
    for i in range(NCORES):
        in_maps.append({
            "frgb": np.ascontiguousarray(frgb_sh[i]),
            "fd": np.ascontiguousarray(fd_sh[i]),
            "wqt": wqt, "wkt": wkt, "wvt": wvt,
            "bq2": bq2, "bk2": bk2, "bvr": bvr,
            "u2a": u2a, "id64": id64, "ones64": ones64, "id2": id2,
        })
    return in_maps, blend


def _execute(in_maps, blend=True, **kwargs):
    key = f"nc_{blend}"
    if key not in _CACHE:
        _CACHE[key] = _build_program(blend)
    res = run_bass_kernel_spmd(_CACHE[key], in_maps, list(range(NCORES)),
                               **kwargs)
    parts = [res.results[i]["out"].astype(np.float32).reshape(BPC, C, H, W)
             for i in range(NCORES)]
    return np.concatenate(parts, axis=0), res


def kernel(F_rgb, F_d, Wq, bq, Wk, bk, Wv, bv, alpha):
    in_maps, blend = _prepare_in_maps(F_rgb, F_d, Wq, bq, Wk, bk, Wv, bv,
                                      alpha)
    out, _ = _execute(in_maps, blend=blend)
    return out
